# revision 23
# baseline (speedup 1.0000x reference)
"""GCN message-passing kernel (nn_CARM_90185723281482) for 8 Trainium2 cores.

Computes, for x [2048, 64, 512], adj_weight [64, 64], kernel [512, 64]:
    adj_hat = D^-1/2 A D^-1/2 + I          (degree from row sums of |A|)
    out = BN(elu(adj_hat @ (x @ kernel) + bias))        -> [2048, 64, 64]

Sharding: data-parallel over the batch axis, 256 batches per core.
Per-core dataflow (rows n = (batch, channel) flattened, R = 16384 rows):
  - x ships bf16, host-pre-transposed: xs[p, j, n] = x2d[n, 128 j + p]
  - load block LB = 2048 rows (4 MiB per 2-LB DMA); first LB split small so
    the pipeline starts early
  - stage 1: supportT[d, n] += kern_j.T @ xT_j into one [128, 512] PSUM tile
    (both 512-row halves of a pb stacked on partitions), single drain
  - PE-transpose supportT back to support chunks ssb [n, (gl,t,d)]
  - stage 2 TRANSPOSED: zT[f, n] = ssb_chunk.T @ a2t per 128-wide f-chunk,
    so d = partition % 64 — all BN/bias constants become per-partition
    scalars riding the ACT bias/scale ports and TensorScalarPtr operands
  - epilogue (a = gamma*rsqrt(var+eps) folded into the stage-1 kernel when
    a > 0, so z_a = a*z comes off the PE):
        q = exp(inv_a*z_a + bias + ln a)  = a*exp(z+bias)      [ACT]
        r = relu(z_a + a*bias)            = a*relu(z+bias)     [ACT]
        t = min(q, a) + (b2 - a)                               [DVE ts]
        out = t + r                                            [Pool tt]
    which equals a*elu(z+bias) + b2 on both branches.
  - output stored transposed [f, n]; host un-permutes.
"""

import sys

import numpy as np

sys.path.insert(0, "/opt/trn_rl_repo")

import concourse.bass as bass  # noqa: E402
from concourse import bacc, bass_utils, mybir, tile  # noqa: E402

F32 = mybir.dt.float32
BF16 = mybir.dt.bfloat16
AF = mybir.ActivationFunctionType
OP = mybir.AluOpType

NCORES = 8
B_FULL, C, Fdim, D = 2048, 64, 512, 64
R = (B_FULL // NCORES) * C  # 16384 rows per core
LB_ROWS = 2048              # rows per load block
NLB = R // LB_ROWS          # 8 load blocks
BN_EPS = 1e-3

_NC_CACHE = {}

# Scheduling/balance knobs
CFG = {
    "px": 3,
    "psT_sb": 4,
    "ps_sb": 3,
    "pep": 3,
    "psT_ps": 3,
    "ps_ps": 3,
    "po_ps": 2,
    "sT_split": 1,         # stage-1 PSUM: 0 = one [128,512], 1 = two [64,512]
    "ssb_split": 0,        # support drain: 0 = whole, 1 = per 256-col half
    "sT_engine": ["act", "dve"],   # supportT drain engine (per gl)
    "ssb_engine": ["dve", "act"],  # support drain engine (per pb)
    "r_engine": "act",     # relu: "act" | "dve" | "pool"
    "t_engine": "dve",     # min/add tensor_scalar
    "add_engine": "dve",   # final add: "pool" | "dve"
    "store_lbs": 4,
    "tailsplit": 1,
    "load_lbs": 2,
    "split_last_store": 1,
    "warmup_mm": 0,        # dummy matmuls (reading cstb) to ramp the PE
    "warmup_act": 0,       # dummy Exp to preload the ACT table early
    "head_pieces": (256, 256, 512, 1024),
}


def _pick(v, pb):
    """Engine knob: either a name or a [pb0, pb1] alternation list."""
    return v[pb % len(v)] if isinstance(v, (list, tuple)) else v


def to_bf16(a):
    """fp32 -> bf16 (RNE), returned as a uint16 array (raw bf16 bits)."""
    u = np.ascontiguousarray(a, np.float32).view(np.uint32).astype(np.uint64)
    r = (u + 0x7FFF + ((u >> 16) & 1)) >> 16
    return r.astype(np.uint16)


def _build_nc(loop_reps=None, variant="ln"):
    nc = bacc.Bacc(
        "TRN2", target_bir_lowering=False, debug=False, num_devices=NCORES
    )
    xs_d = nc.dram_tensor("xs", [Fdim, R], BF16, kind="ExternalInput").ap()
    cstb_d = nc.dram_tensor("cstb", [128, 512], BF16, kind="ExternalInput").ap()
    cst2_d = nc.dram_tensor("cst2", [128, 8], F32, kind="ExternalInput").ap()
    out_d = nc.dram_tensor("out", [128, (R // 128) * D], BF16,
                           kind="ExternalOutput").ap()

    with tile.TileContext(nc) as tc, \
         tc.tile_pool(name="consts", bufs=1) as consts, \
         tc.tile_pool(name="px", bufs=CFG["px"]) as px, \
         tc.tile_pool(name="psT_ps", bufs=CFG["psT_ps"], space="PSUM") as psT_ps, \
         tc.tile_pool(name="psT_sb", bufs=CFG["psT_sb"]) as psT_sb, \
         tc.tile_pool(name="ps_ps", bufs=CFG["ps_ps"], space="PSUM") as ps_ps, \
         tc.tile_pool(name="ps_sb", bufs=CFG["ps_sb"]) as ps_sb, \
         tc.tile_pool(name="po_ps", bufs=CFG["po_ps"], space="PSUM") as po_ps, \
         tc.tile_pool(name="pep", bufs=CFG["pep"]) as pep, \
         tc.tile_pool(name="pout", bufs=2) as pout:

        cstb = consts.tile([128, 512], BF16, tag="cstb")
        nc.sync.dma_start(cstb[:], cstb_d)
        cst2 = consts.tile([128, 8], F32, tag="cst2")
        identr = cstb[:, 0:128]
        kern = cstb[:, 128:384]
        a2t = cstb[:, 384:512]
        inv_a = cst2[:, 0:1]
        bias_exp = cst2[:, 1:2]
        rbias = cst2[:, 2:3]
        a_col = cst2[:, 3:4]
        b3_col = cst2[:, 4:5]
        b2_col = cst2[:, 5:6]

        # PSUM->SBUF drains and element ops with an engine choice.
        def drain(dst_ap, src_ap, eng):
            if eng == "act":
                nc.scalar.activation(dst_ap, src_ap, AF.Copy)
            else:
                nc.vector.tensor_copy(dst_ap, src_ap)

        # Warmup: the PE runs at 0.65/1.2 GHz until ~3us of continuous
        # execution, and the first Activation pays a 1.3us table load.
        # Burn both on dummy reads of cstb while the first x pieces are
        # still in flight, so real work starts at full speed.
        if CFG["warmup_act"]:
            wact = consts.tile([128, 8], F32, tag="wact")
            nc.scalar.activation(wact[:], cstb[:, 0:8], AF.Exp)
        if CFG["warmup_mm"]:
            with tc.tile_pool(name="pwarm", bufs=1, space="PSUM") as pwarm:
                wps = pwarm.tile([64, 512], F32, tag="w")
                for _ in range(CFG["warmup_mm"]):
                    nc.tensor.matmul(wps[:], cstb[:, 0:64], cstb[:, 0:512],
                                     start=True, stop=True)

        import contextlib
        loop_cm = tc.For_i(0, loop_reps, 1) if loop_reps else \
            contextlib.nullcontext()
        with loop_cm:
            _body(nc, tc, locals(), variant)
    nc.compile()
    return nc


def _body(nc, tc, env, variant):
    px = env["px"]
    psT_ps, psT_sb = env["psT_ps"], env["psT_sb"]
    ps_ps, ps_sb, po_ps = env["ps_ps"], env["ps_sb"], env["po_ps"]
    pep, pout = env["pep"], env["pout"]
    xs_d, out_d, cst2_d = env["xs_d"], env["out_d"], env["cst2_d"]
    kern, identr, a2t = env["kern"], env["identr"], env["a2t"]
    inv_a, bias_exp, rbias = env["inv_a"], env["bias_exp"], env["rbias"]
    a_col, b3_col, b2_col = env["a_col"], env["b3_col"], env["b2_col"]
    cst2 = env["cst2"]
    drain = env["drain"]

    def ts(eng, *a, **k):
        (nc.vector if eng == "dve" else nc.gpsimd).tensor_scalar(*a, **k)

    def tt_add(eng, out, x, y):
        if eng == "dve":
            nc.vector.tensor_add(out, x, y)
        else:
            nc.gpsimd.tensor_add(out, x, y)

    xsT_v = xs_d.rearrange("(j p) n -> p j n", p=128)
    LL = CFG["load_lbs"]
    SL = CFG["store_lbs"]
    for lb in range(NLB):
        if lb % LL == 0:
            xsb = px.tile([128, 4 * LL * LB_ROWS], BF16, tag="x")
            xsb_v = xsb[:].rearrange("p (j n) -> p j n", j=4)
            # Split the first/last loads so compute starts early
            if lb == 0:
                pieces = list(CFG["head_pieces"])
                rest = LL * LB_ROWS - sum(pieces)
                pieces += [rest] if rest else []
            elif lb == NLB - LL and CFG["tailsplit"]:
                pieces = [1024] * (LL * LB_ROWS // 1024)
            else:
                pieces = [LL * LB_ROWS]
            n0 = 0
            for pi, pn in enumerate(pieces):
                nc.sync.dma_start(
                    xsb_v[:, :, n0:n0 + pn],
                    xsT_v[:, :, lb * LB_ROWS + n0:lb * LB_ROWS + n0 + pn],
                )
                n0 += pn
                if lb == 0 and pi == 0:
                    # tiny f32 constant columns; issued after the first x
                    # piece so they don't delay the pipeline start
                    nc.sync.dma_start(cst2[:], cst2_d)
        nw0 = (lb % LL) * LB_ROWS
        if lb % SL == 0:
            outsb = pout.tile([128, SL * 2 * 512], BF16, tag="out")
        for pb in range(2):
            # stage 1: supportT [d, n]; one [128,512] tile or two [64,512]
            if CFG["sT_split"]:
                sT_views = []
                for gl in range(2):
                    g = 2 * pb + gl
                    sTps = psT_ps.tile([64, 512], F32, tag="sTp")
                    for j in range(4):
                        nc.tensor.matmul(
                            sTps[:],
                            kern[:, 64 * j:64 * (j + 1)],
                            xsb_v[:, j, nw0 + 512 * g:nw0 + 512 * (g + 1)],
                            start=(j == 0),
                            stop=(j == 3),
                        )
                    sTsb = psT_sb.tile([64, 512], BF16, tag="sTs")
                    drain(sTsb[:], sTps[:],
                          _pick(CFG["sT_engine"], 2 * pb + gl))
                    sT_views.append((sTsb, 0))
            else:
                sTps = psT_ps.tile([128, 512], F32, tag="sTp")
                for gl in range(2):
                    g = 2 * pb + gl
                    for j in range(4):
                        nc.tensor.matmul(
                            sTps[64 * gl:64 * (gl + 1), :],
                            kern[:, 64 * j:64 * (j + 1)],
                            xsb_v[:, j, nw0 + 512 * g:nw0 + 512 * (g + 1)],
                            start=(j == 0),
                            stop=(j == 3),
                        )
                sTsb = psT_sb.tile([128, 512], BF16, tag="sTs")
                drain(sTsb[:], sTps[:], _pick(CFG["sT_engine"], pb))
                sT_views = [(sTsb, 0), (sTsb, 64)]
            # transpose supportT -> support chunks [n, (gl,t,d)], then
            # drain + stage 2 per half so halves pipeline
            ssb = ps_sb.tile([128, 512], BF16, tag="ss")
            zps = po_ps.tile([128, 512], F32, tag="op")
            if not CFG["ssb_split"]:
                sps = ps_ps.tile([128, 512], BF16, tag="sp")
            for gl in range(2):
                src, p0 = sT_views[gl]
                ident = identr[p0:p0 + 64, p0:p0 + 64] if p0 else \
                    identr[:64, :64]
                if CFG["ssb_split"]:
                    # per-gl [128,256] PSUM tile: half the bank footprint
                    sps_g = ps_ps.tile([128, 256], BF16, tag="sp")
                    for t in range(4):
                        nc.tensor.transpose(
                            sps_g[:, 64 * t:64 * (t + 1)],
                            src[p0:p0 + 64, 128 * t:128 * (t + 1)],
                            ident,
                        )
                    h0 = 256 * gl
                    drain(ssb[:, h0:h0 + 256], sps_g[:],
                          _pick(CFG["ssb_engine"], 2 * pb + gl))
                    for m in (2 * gl, 2 * gl + 1):
                        nc.tensor.matmul(
                            zps[:, 128 * m:128 * (m + 1)],
                            ssb[:, 128 * m:128 * (m + 1)],
                            a2t, start=True, stop=True,
                        )
                else:
                    for t in range(4):
                        nc.tensor.transpose(
                            sps[:, 256 * gl + 64 * t:256 * gl + 64 * (t + 1)],
                            src[p0:p0 + 64, 128 * t:128 * (t + 1)],
                            ident,
                        )
            if not CFG["ssb_split"]:
                drain(ssb[:], sps[:], _pick(CFG["ssb_engine"], pb))
                for m in range(4):
                    nc.tensor.matmul(
                        zps[:, 128 * m:128 * (m + 1)],
                        ssb[:, 128 * m:128 * (m + 1)],
                        a2t, start=True, stop=True,
                    )
            # epilogue: per-partition constants (d = partition % 64)
            ob = 1024 * (lb % SL) + 512 * pb
            q = pep.tile([128, 512], BF16, tag="q")
            nc.scalar.activation(q[:], zps[:], AF.Exp,
                                 bias=bias_exp, scale=inv_a)
            if variant == "ln":
                # q = a*exp(y), y = z+bias. out = a*elu(y) + b2
                #   = max(a*y + b2, min(q, a) + (b2-a))  [y <= e^y - 1]
                # with a*y = z_a + a*bias coming straight off the PE.
                t1 = pep.tile([128, 512], BF16, tag="t")
                ts(_pick(CFG["t_engine"], pb), t1[:], q[:], a_col, b3_col,
                   OP.min, OP.add)
                stt_eng = _pick(CFG["add_engine"], pb)
                (nc.vector if stt_eng == "dve" else
                 nc.gpsimd).scalar_tensor_tensor(
                    outsb[:, ob:ob + 512], zps[:], rbias, t1[:],
                    OP.add, OP.max)
            else:
                r = pep.tile([128, 512], BF16, tag="r")
                if _pick(CFG["r_engine"], pb) == "act":
                    nc.scalar.activation(r[:], zps[:], AF.Relu, bias=rbias)
                else:
                    ts(_pick(CFG["r_engine"], pb), r[:], zps[:], rbias, 0.0,
                       OP.add, OP.max)
                # safe for a<=0: q=exp(z+bias), r=relu(z+bias);
                # elu = r + min(q-1, 0); out = a*elu + b2
                t1 = pep.tile([128, 512], BF16, tag="t")
                ts(_pick(CFG["t_engine"], pb), t1[:], q[:], 1.0, 0.0,
                   OP.subtract, OP.min)
                s1 = pep.tile([128, 512], BF16, tag="s")
                tt_add(_pick(CFG["add_engine"], pb), s1[:], t1[:], r[:])
                ts("dve", outsb[:, ob:ob + 512], s1[:], a_col, b2_col,
                   OP.mult, OP.add)
        if lb % SL == SL - 1:
            # out DRAM is partition-major; host un-permutes
            c0 = (lb - SL + 1) * 2 * 512
            if lb == NLB - 1 and CFG["split_last_store"]:
                for h in range(SL):
                    nc.sync.dma_start(
                        out_d[:, c0 + h * 1024:c0 + (h + 1) * 1024],
                        outsb[:, h * 1024:(h + 1) * 1024],
                    )
            else:
                nc.sync.dma_start(
                    out_d[:, c0:c0 + SL * 1024], outsb[:],
                )


def get_nc(variant="ln"):
    if variant not in _NC_CACHE:
        _NC_CACHE[variant] = _build_nc(variant=variant)
    return _NC_CACHE[variant]


def host_prep(inputs):
    adj = np.asarray(inputs["adj_weight"], np.float32)
    kern = np.ascontiguousarray(np.asarray(inputs["kernel"], np.float32))
    bias = np.asarray(inputs["bias"], np.float32)
    gamma = np.asarray(inputs["gamma"], np.float32)
    beta = np.asarray(inputs["beta"], np.float32)
    mm = np.asarray(inputs["moving_mean"], np.float32)
    mv = np.asarray(inputs["moving_var"], np.float32)

    deg = np.maximum(np.abs(adj).sum(axis=1, keepdims=True), 1e-8)
    dis = deg ** -0.5
    adj_hat = adj * dis * dis.T + np.eye(C, dtype=np.float32)
    a2t = np.zeros((128, 128), np.float32)
    a2t[:64, :64] = adj_hat.T
    a2t[64:, 64:] = adj_hat.T

    a = (gamma / np.sqrt(mv + BN_EPS)).astype(np.float32)
    b2 = (beta - mm * a).astype(np.float32)
    variant = "ln" if np.all(a > 0) else "safe"

    # kern laid out [128, j, d]: kern_sb[p, j, d] = kernel[128 j + p, d],
    # with the BN scale folded in on the ln path
    kern_f = kern * a[None, :] if variant == "ln" else kern
    kern_t = kern_f.reshape(4, 128, D).transpose(1, 0, 2).reshape(128, 4 * D)

    cstb = np.zeros((128, 512), np.float32)
    cstb[:, 0:128] = np.eye(128, dtype=np.float32)
    cstb[:, 128:384] = kern_t
    cstb[:, 384:512] = a2t
    cstb = to_bf16(cstb)

    # per-partition constant columns: d = partition % 64
    dd = np.arange(128) % 64
    cst2 = np.zeros((128, 8), np.float32)
    if variant == "ln":
        cst2[:, 0] = (1.0 / a)[dd]
        cst2[:, 1] = (bias + np.log(a))[dd]
        cst2[:, 2] = (a * bias + b2)[dd]
    else:
        cst2[:, 0] = 1.0
        cst2[:, 1] = bias[dd]
        cst2[:, 2] = bias[dd]
    cst2[:, 3] = a[dd]
    cst2[:, 4] = (b2 - a)[dd]
    cst2[:, 5] = b2[dd]

    x = np.asarray(inputs["x"], np.float32)
    shards = x.reshape(NCORES, R, Fdim)
    import ml_dtypes
    in_maps = [
        {
            "xs": np.ascontiguousarray(to_bf16(shards[i]).T)
                  .view(ml_dtypes.bfloat16),
            "cstb": cstb.view(ml_dtypes.bfloat16),
            "cst2": cst2,
        }
        for i in range(NCORES)
    ]
    return in_maps, variant


def run(inputs, trace=False, **kw):
    in_maps, variant = host_prep(inputs)
    nc = get_nc(variant)
    try:
        res = bass_utils.run_bass_kernel_spmd(
            nc, in_maps, core_ids=list(range(NCORES)), trace=trace, **kw
        )
    except Exception:
        # transient NRT_EXEC_UNIT_UNRECOVERABLE has been observed right
        # after a previous process's teardown; one retry clears it
        import time as _time
        _time.sleep(5.0)
        res = bass_utils.run_bass_kernel_spmd(
            nc, in_maps, core_ids=list(range(NCORES)), trace=trace, **kw
        )
    shards = []
    for i in range(NCORES):
        raw = np.asarray(res.results[i]["out"]).astype(np.float32)
        # raw[p, C]: C = pbg*512 + 128*(2*gl+tq) + 64*h + c,
        # p = 64*ph + d; n = pbg*1024 + gl*512 + (2*tq+ph)*128 + 64*h + c
        shards.append(
            raw.reshape(2, 64, 16, 2, 2, 2, 64)
               .transpose(2, 3, 4, 0, 5, 6, 1)
               .reshape(R, D)
        )
    out = np.concatenate(shards, axis=0).reshape(B_FULL, C, D)
    return out, res


def kernel(**inputs) -> np.ndarray:
    out, _ = run(inputs)
    return out


# revision 27
# speedup vs baseline: 1.0333x; 1.0333x over previous
"""GCN message-passing kernel (nn_CARM_90185723281482) for 8 Trainium2 cores.

Computes, for x [2048, 64, 512], adj_weight [64, 64], kernel [512, 64]:
    adj_hat = D^-1/2 A D^-1/2 + I          (degree from row sums of |A|)
    out = BN(elu(adj_hat @ (x @ kernel) + bias))        -> [2048, 64, 64]

Sharding: data-parallel over the batch axis, 256 batches per core.
Per-core dataflow (rows n = (batch, channel) flattened, R = 16384 rows):
  - x ships bf16, host-pre-transposed: xs[p, j, n] = x2d[n, 128 j + p]
  - load block LB = 2048 rows (4 MiB per 2-LB DMA); first LB split small so
    the pipeline starts early
  - stage 1: supportT[d, n] += kern_j.T @ xT_j into one [128, 512] PSUM tile
    (both 512-row halves of a pb stacked on partitions), single drain
  - PE-transpose supportT back to support chunks ssb [n, (gl,t,d)]
  - stage 2 TRANSPOSED: zT[f, n] = ssb_chunk.T @ a2t per 128-wide f-chunk,
    so d = partition % 64 — all BN/bias constants become per-partition
    scalars riding the ACT bias/scale ports and TensorScalarPtr operands
  - epilogue (a = gamma*rsqrt(var+eps) folded into the stage-1 kernel when
    a > 0, so z_a = a*z comes off the PE):
        q = exp(inv_a*z_a + bias + ln a)  = a*exp(z+bias)      [ACT]
        r = relu(z_a + a*bias)            = a*relu(z+bias)     [ACT]
        t = min(q, a) + (b2 - a)                               [DVE ts]
        out = t + r                                            [Pool tt]
    which equals a*elu(z+bias) + b2 on both branches.
  - output stored transposed [f, n]; host un-permutes.
"""

import sys

import numpy as np

sys.path.insert(0, "/opt/trn_rl_repo")

import concourse.bass as bass  # noqa: E402
from concourse import bacc, bass_utils, mybir, tile  # noqa: E402

F32 = mybir.dt.float32
BF16 = mybir.dt.bfloat16
AF = mybir.ActivationFunctionType
OP = mybir.AluOpType

NCORES = 8
B_FULL, C, Fdim, D = 2048, 64, 512, 64
R = (B_FULL // NCORES) * C  # 16384 rows per core
LB_ROWS = 2048              # rows per load block
NLB = R // LB_ROWS          # 8 load blocks
BN_EPS = 1e-3

_NC_CACHE = {}

# Scheduling/balance knobs (tuned against the TimelineSim cost model)
CFG = {
    "px": 6,
    "psT_sb": 6,
    "ps_sb": 3,
    "pep": 4,
    "psT_ps": 3,
    "ps_ps": 2,
    "po_ps": 3,
    "sT_split": 1,         # stage-1 PSUM: 0 = one [128,512], 1 = two [64,512]
    "ssb_split": 0,        # support drain: 0 = whole, 1 = per 256-col half
    "sT_engine": ["act", "dve"],   # supportT drain engine (per gl)
    "ssb_engine": ["dve", "act"],  # support drain engine (per pb)
    "r_engine": "act",     # relu (safe variant): "act" | "dve" | "pool"
    "t_engine": "dve",     # min/add tensor_scalar
    "add_engine": "dve",   # final scalar_tensor_tensor / add
    "store_lbs": 4,
    "tailsplit": 1,
    "load_lbs": 1,
    "split_last_store": 1,
    "warmup_mm": 0,        # dummy matmuls (reading cstb) to ramp the PE
    "warmup_act": 0,       # dummy Exp to preload the ACT table early
    "head_pieces": (1024,),
    "ep_split": 0,         # epilogue per 256-col half
}


def _pick(v, pb):
    """Engine knob: either a name or a [pb0, pb1] alternation list."""
    return v[pb % len(v)] if isinstance(v, (list, tuple)) else v


def to_bf16(a):
    """fp32 -> bf16 (RNE), returned as a uint16 array (raw bf16 bits)."""
    u = np.ascontiguousarray(a, np.float32).view(np.uint32).astype(np.uint64)
    r = (u + 0x7FFF + ((u >> 16) & 1)) >> 16
    return r.astype(np.uint16)


def _build_nc(loop_reps=None, variant="ln"):
    nc = bacc.Bacc(
        "TRN2", target_bir_lowering=False, debug=False, num_devices=NCORES
    )
    xs_d = nc.dram_tensor("xs", [Fdim, R], BF16, kind="ExternalInput").ap()
    cstb_d = nc.dram_tensor("cstb", [128, 512], BF16, kind="ExternalInput").ap()
    cst2_d = nc.dram_tensor("cst2", [128, 8], F32, kind="ExternalInput").ap()
    out_d = nc.dram_tensor("out", [128, (R // 128) * D], BF16,
                           kind="ExternalOutput").ap()

    with tile.TileContext(nc) as tc, \
         tc.tile_pool(name="consts", bufs=1) as consts, \
         tc.tile_pool(name="px", bufs=CFG["px"]) as px, \
         tc.tile_pool(name="psT_ps", bufs=CFG["psT_ps"], space="PSUM") as psT_ps, \
         tc.tile_pool(name="psT_sb", bufs=CFG["psT_sb"]) as psT_sb, \
         tc.tile_pool(name="ps_ps", bufs=CFG["ps_ps"], space="PSUM") as ps_ps, \
         tc.tile_pool(name="ps_sb", bufs=CFG["ps_sb"]) as ps_sb, \
         tc.tile_pool(name="po_ps", bufs=CFG["po_ps"], space="PSUM") as po_ps, \
         tc.tile_pool(name="pep", bufs=CFG["pep"]) as pep, \
         tc.tile_pool(name="pout", bufs=2) as pout:

        cstb = consts.tile([128, 512], BF16, tag="cstb")
        nc.sync.dma_start(cstb[:], cstb_d)
        cst2 = consts.tile([128, 8], F32, tag="cst2")
        identr = cstb[:, 0:128]
        kern = cstb[:, 128:384]
        a2t = cstb[:, 384:512]
        inv_a = cst2[:, 0:1]
        bias_exp = cst2[:, 1:2]
        rbias = cst2[:, 2:3]
        a_col = cst2[:, 3:4]
        b3_col = cst2[:, 4:5]
        b2_col = cst2[:, 5:6]

        # PSUM->SBUF drains and element ops with an engine choice.
        def drain(dst_ap, src_ap, eng):
            if eng == "act":
                nc.scalar.activation(dst_ap, src_ap, AF.Copy)
            else:
                nc.vector.tensor_copy(dst_ap, src_ap)

        # Warmup: the PE runs at 0.65/1.2 GHz until ~3us of continuous
        # execution, and the first Activation pays a 1.3us table load.
        # Burn both on dummy reads of cstb while the first x pieces are
        # still in flight, so real work starts at full speed.
        if CFG["warmup_act"]:
            wact = consts.tile([128, 8], F32, tag="wact")
            nc.scalar.activation(wact[:], cstb[:, 0:8], AF.Exp)
        if CFG["warmup_mm"]:
            with tc.tile_pool(name="pwarm", bufs=1, space="PSUM") as pwarm:
                wps = pwarm.tile([64, 512], F32, tag="w")
                for _ in range(CFG["warmup_mm"]):
                    nc.tensor.matmul(wps[:], cstb[:, 0:64], cstb[:, 0:512],
                                     start=True, stop=True)

        import contextlib
        loop_cm = tc.For_i(0, loop_reps, 1) if loop_reps else \
            contextlib.nullcontext()
        with loop_cm:
            _body(nc, tc, locals(), variant)
    nc.compile()
    return nc


def _body(nc, tc, env, variant):
    px = env["px"]
    psT_ps, psT_sb = env["psT_ps"], env["psT_sb"]
    ps_ps, ps_sb, po_ps = env["ps_ps"], env["ps_sb"], env["po_ps"]
    pep, pout = env["pep"], env["pout"]
    xs_d, out_d, cst2_d = env["xs_d"], env["out_d"], env["cst2_d"]
    kern, identr, a2t = env["kern"], env["identr"], env["a2t"]
    inv_a, bias_exp, rbias = env["inv_a"], env["bias_exp"], env["rbias"]
    a_col, b3_col, b2_col = env["a_col"], env["b3_col"], env["b2_col"]
    cst2 = env["cst2"]
    drain = env["drain"]

    def ts(eng, *a, **k):
        (nc.vector if eng == "dve" else nc.gpsimd).tensor_scalar(*a, **k)

    def tt_add(eng, out, x, y):
        if eng == "dve":
            nc.vector.tensor_add(out, x, y)
        else:
            nc.gpsimd.tensor_add(out, x, y)

    xsT_v = xs_d.rearrange("(j p) n -> p j n", p=128)
    LL = CFG["load_lbs"]
    SL = CFG["store_lbs"]
    for lb in range(NLB):
        if lb % LL == 0:
            xsb = px.tile([128, 4 * LL * LB_ROWS], BF16, tag="x")
            xsb_v = xsb[:].rearrange("p (j n) -> p j n", j=4)
            # Split the first/last loads so compute starts early
            if lb == 0:
                pieces = list(CFG["head_pieces"])
                rest = LL * LB_ROWS - sum(pieces)
                pieces += [rest] if rest else []
            elif lb == NLB - LL and CFG["tailsplit"]:
                pieces = [1024] * (LL * LB_ROWS // 1024)
            else:
                pieces = [LL * LB_ROWS]
            n0 = 0
            for pi, pn in enumerate(pieces):
                nc.sync.dma_start(
                    xsb_v[:, :, n0:n0 + pn],
                    xsT_v[:, :, lb * LB_ROWS + n0:lb * LB_ROWS + n0 + pn],
                )
                n0 += pn
                if lb == 0 and pi == 0:
                    # tiny f32 constant columns; issued after the first x
                    # piece so they don't delay the pipeline start
                    nc.sync.dma_start(cst2[:], cst2_d)
        nw0 = (lb % LL) * LB_ROWS
        if lb % SL == 0:
            outsb = pout.tile([128, SL * 2 * 512], BF16, tag="out")
        for pb in range(2):
            # stage 1: supportT [d, n]; one [128,512] tile or two [64,512]
            if CFG["sT_split"]:
                sT_views = []
                for gl in range(2):
                    g = 2 * pb + gl
                    sTps = psT_ps.tile([64, 512], F32, tag="sTp")
                    for j in range(4):
                        nc.tensor.matmul(
                            sTps[:],
                            kern[:, 64 * j:64 * (j + 1)],
                            xsb_v[:, j, nw0 + 512 * g:nw0 + 512 * (g + 1)],
                            start=(j == 0),
                            stop=(j == 3),
                        )
                    sTsb = psT_sb.tile([64, 512], BF16, tag="sTs")
                    drain(sTsb[:], sTps[:],
                          _pick(CFG["sT_engine"], 2 * pb + gl))
                    sT_views.append((sTsb, 0))
            else:
                sTps = psT_ps.tile([128, 512], F32, tag="sTp")
                for gl in range(2):
                    g = 2 * pb + gl
                    for j in range(4):
                        nc.tensor.matmul(
                            sTps[64 * gl:64 * (gl + 1), :],
                            kern[:, 64 * j:64 * (j + 1)],
                            xsb_v[:, j, nw0 + 512 * g:nw0 + 512 * (g + 1)],
                            start=(j == 0),
                            stop=(j == 3),
                        )
                sTsb = psT_sb.tile([128, 512], BF16, tag="sTs")
                drain(sTsb[:], sTps[:], _pick(CFG["sT_engine"], pb))
                sT_views = [(sTsb, 0), (sTsb, 64)]
            # transpose supportT -> support chunks [n, (gl,t,d)], then
            # drain + stage 2 per half so halves pipeline
            ssb = ps_sb.tile([128, 512], BF16, tag="ss")
            zps = po_ps.tile([128, 512], F32, tag="op")
            if not CFG["ssb_split"]:
                sps = ps_ps.tile([128, 512], BF16, tag="sp")
            for gl in range(2):
                src, p0 = sT_views[gl]
                ident = identr[p0:p0 + 64, p0:p0 + 64] if p0 else \
                    identr[:64, :64]
                if CFG["ssb_split"]:
                    # per-gl [128,256] PSUM tile: half the bank footprint
                    sps_g = ps_ps.tile([128, 256], BF16, tag="sp")
                    for t in range(4):
                        nc.tensor.transpose(
                            sps_g[:, 64 * t:64 * (t + 1)],
                            src[p0:p0 + 64, 128 * t:128 * (t + 1)],
                            ident,
                        )
                    h0 = 256 * gl
                    drain(ssb[:, h0:h0 + 256], sps_g[:],
                          _pick(CFG["ssb_engine"], 2 * pb + gl))
                    for m in (2 * gl, 2 * gl + 1):
                        nc.tensor.matmul(
                            zps[:, 128 * m:128 * (m + 1)],
                            ssb[:, 128 * m:128 * (m + 1)],
                            a2t, start=True, stop=True,
                        )
                else:
                    for t in range(4):
                        nc.tensor.transpose(
                            sps[:, 256 * gl + 64 * t:256 * gl + 64 * (t + 1)],
                            src[p0:p0 + 64, 128 * t:128 * (t + 1)],
                            ident,
                        )
            if not CFG["ssb_split"]:
                drain(ssb[:], sps[:], _pick(CFG["ssb_engine"], pb))
                for m in range(4):
                    nc.tensor.matmul(
                        zps[:, 128 * m:128 * (m + 1)],
                        ssb[:, 128 * m:128 * (m + 1)],
                        a2t, start=True, stop=True,
                    )
            # epilogue: per-partition constants (d = partition % 64)
            ob = 1024 * (lb % SL) + 512 * pb
            if variant == "ln" and CFG["ep_split"]:
                # q = a*exp(y), y = z+bias. out = a*elu(y) + b2
                #   = max(a*y + b2, min(q, a) + (b2-a))  [y <= e^y - 1]
                # per 256-col half so the gl0 half flows while gl1
                # is still transposing
                q = pep.tile([128, 512], BF16, tag="q")
                t1 = pep.tile([128, 512], BF16, tag="t")
                for gl in range(2):
                    h0 = 256 * gl
                    nc.scalar.activation(q[:, h0:h0 + 256],
                                         zps[:, h0:h0 + 256], AF.Exp,
                                         bias=bias_exp, scale=inv_a)
                    ts(_pick(CFG["t_engine"], gl), t1[:, h0:h0 + 256],
                       q[:, h0:h0 + 256], a_col, b3_col, OP.min, OP.add)
                    stt_eng = _pick(CFG["add_engine"], gl)
                    (nc.vector if stt_eng == "dve" else
                     nc.gpsimd).scalar_tensor_tensor(
                        outsb[:, ob + h0:ob + h0 + 256],
                        zps[:, h0:h0 + 256], rbias, t1[:, h0:h0 + 256],
                        OP.add, OP.max)
            elif variant == "ln":
                q = pep.tile([128, 512], BF16, tag="q")
                t1 = pep.tile([128, 512], BF16, tag="t")
                nc.scalar.activation(q[:], zps[:], AF.Exp,
                                     bias=bias_exp, scale=inv_a)
                ts(_pick(CFG["t_engine"], pb), t1[:], q[:], a_col, b3_col,
                   OP.min, OP.add)
                stt_eng = _pick(CFG["add_engine"], pb)
                (nc.vector if stt_eng == "dve" else
                 nc.gpsimd).scalar_tensor_tensor(
                    outsb[:, ob:ob + 512], zps[:], rbias, t1[:],
                    OP.add, OP.max)
            else:
                q = pep.tile([128, 512], BF16, tag="q")
                nc.scalar.activation(q[:], zps[:], AF.Exp,
                                     bias=bias_exp, scale=inv_a)
                r = pep.tile([128, 512], BF16, tag="r")
                if _pick(CFG["r_engine"], pb) == "act":
                    nc.scalar.activation(r[:], zps[:], AF.Relu, bias=rbias)
                else:
                    ts(_pick(CFG["r_engine"], pb), r[:], zps[:], rbias, 0.0,
                       OP.add, OP.max)
                # safe for a<=0: q=exp(z+bias), r=relu(z+bias);
                # elu = r + min(q-1, 0); out = a*elu + b2
                t1 = pep.tile([128, 512], BF16, tag="t")
                ts(_pick(CFG["t_engine"], pb), t1[:], q[:], 1.0, 0.0,
                   OP.subtract, OP.min)
                s1 = pep.tile([128, 512], BF16, tag="s")
                tt_add(_pick(CFG["add_engine"], pb), s1[:], t1[:], r[:])
                ts("dve", outsb[:, ob:ob + 512], s1[:], a_col, b2_col,
                   OP.mult, OP.add)
        if lb % SL == SL - 1:
            # out DRAM is partition-major; host un-permutes
            c0 = (lb - SL + 1) * 2 * 512
            if lb == NLB - 1 and CFG["split_last_store"]:
                for h in range(SL):
                    nc.sync.dma_start(
                        out_d[:, c0 + h * 1024:c0 + (h + 1) * 1024],
                        outsb[:, h * 1024:(h + 1) * 1024],
                    )
            else:
                nc.sync.dma_start(
                    out_d[:, c0:c0 + SL * 1024], outsb[:],
                )


def get_nc(variant="ln"):
    if variant not in _NC_CACHE:
        _NC_CACHE[variant] = _build_nc(variant=variant)
    return _NC_CACHE[variant]


def host_prep(inputs):
    adj = np.asarray(inputs["adj_weight"], np.float32)
    kern = np.ascontiguousarray(np.asarray(inputs["kernel"], np.float32))
    bias = np.asarray(inputs["bias"], np.float32)
    gamma = np.asarray(inputs["gamma"], np.float32)
    beta = np.asarray(inputs["beta"], np.float32)
    mm = np.asarray(inputs["moving_mean"], np.float32)
    mv = np.asarray(inputs["moving_var"], np.float32)

    deg = np.maximum(np.abs(adj).sum(axis=1, keepdims=True), 1e-8)
    dis = deg ** -0.5
    adj_hat = adj * dis * dis.T + np.eye(C, dtype=np.float32)
    a2t = np.zeros((128, 128), np.float32)
    a2t[:64, :64] = adj_hat.T
    a2t[64:, 64:] = adj_hat.T

    a = (gamma / np.sqrt(mv + BN_EPS)).astype(np.float32)
    b2 = (beta - mm * a).astype(np.float32)
    variant = "ln" if np.all(a > 0) else "safe"

    # kern laid out [128, j, d]: kern_sb[p, j, d] = kernel[128 j + p, d],
    # with the BN scale folded in on the ln path
    kern_f = kern * a[None, :] if variant == "ln" else kern
    kern_t = kern_f.reshape(4, 128, D).transpose(1, 0, 2).reshape(128, 4 * D)

    cstb = np.zeros((128, 512), np.float32)
    cstb[:, 0:128] = np.eye(128, dtype=np.float32)
    cstb[:, 128:384] = kern_t
    cstb[:, 384:512] = a2t
    cstb = to_bf16(cstb)

    # per-partition constant columns: d = partition % 64
    dd = np.arange(128) % 64
    cst2 = np.zeros((128, 8), np.float32)
    if variant == "ln":
        cst2[:, 0] = (1.0 / a)[dd]
        cst2[:, 1] = (bias + np.log(a))[dd]
        cst2[:, 2] = (a * bias + b2)[dd]
    else:
        cst2[:, 0] = 1.0
        cst2[:, 1] = bias[dd]
        cst2[:, 2] = bias[dd]
    cst2[:, 3] = a[dd]
    cst2[:, 4] = (b2 - a)[dd]
    cst2[:, 5] = b2[dd]

    x = np.asarray(inputs["x"], np.float32)
    shards = x.reshape(NCORES, R, Fdim)
    import ml_dtypes
    in_maps = [
        {
            "xs": np.ascontiguousarray(to_bf16(shards[i]).T)
                  .view(ml_dtypes.bfloat16),
            "cstb": cstb.view(ml_dtypes.bfloat16),
            "cst2": cst2,
        }
        for i in range(NCORES)
    ]
    return in_maps, variant


def run(inputs, trace=False, **kw):
    in_maps, variant = host_prep(inputs)
    nc = get_nc(variant)
    try:
        res = bass_utils.run_bass_kernel_spmd(
            nc, in_maps, core_ids=list(range(NCORES)), trace=trace, **kw
        )
    except Exception:
        # transient NRT_EXEC_UNIT_UNRECOVERABLE has been observed right
        # after a previous process's teardown; one retry clears it
        import time as _time
        _time.sleep(5.0)
        res = bass_utils.run_bass_kernel_spmd(
            nc, in_maps, core_ids=list(range(NCORES)), trace=trace, **kw
        )
    shards = []
    for i in range(NCORES):
        raw = np.asarray(res.results[i]["out"]).astype(np.float32)
        # raw[p, C]: C = pbg*512 + 128*(2*gl+tq) + 64*h + c,
        # p = 64*ph + d; n = pbg*1024 + gl*512 + (2*tq+ph)*128 + 64*h + c
        shards.append(
            raw.reshape(2, 64, 16, 2, 2, 2, 64)
               .transpose(2, 3, 4, 0, 5, 6, 1)
               .reshape(R, D)
        )
    out = np.concatenate(shards, axis=0).reshape(B_FULL, C, D)
    return out, res


def kernel(**inputs) -> np.ndarray:
    out, _ = run(inputs)
    return out
